# revision 39
# baseline (speedup 1.0000x reference)
"""LoRA Linear (residual + low-rank path with dropout) on 8 Trainium2 cores.

Math (fp32 reference):
  residual = hidden_states @ W_base.T
  dropped  = hidden_states * dropout_mask / (1 - p)
  out      = residual + ((dropped @ A.T) @ B.T) * scaling

Sharding: data-parallel over the 8192 tokens (8 cores x 1024 tokens);
W_base / A / B replicated.

v12 — token-split two-pass structure. The host ships x pre-cast to
fp8-e4m3 (4MB/core) and the dropout product d = bf16(x)*mask in bf16
(8MB/core, replacing the u8 mask + on-device DVE cast/mult). The residual
matmul runs fp8 DoubleRow (W pre-scaled by 64, drain rescales 1/64); the
LoRA path (A', B', d) stays bf16 — fp8 anywhere on the LoRA path fails
the 2e-2 gate (measured 3.2e-2 in numpy). Output is written bf16 (halves
drain traffic; adds <1e-3 to a ~5e-3 error vs the 2e-2 gate).

The 1024 tokens split into two 512-token halves so each PSUM chain is one
[128, 512] bank:
  Pass A (h0): per-k stream of x8/d rides 7 prologue chunks' DR chains
    (one bank each) + stage-1(h0) (8th bank). Then 24 steady chunks in
    PAIRS; h1's x8/d + stage-1(h1) interleave, DMA one step ahead of the
    consuming matmul.
  Pass B (h1): all 32 chunks with ZERO input DMA — W (100KB/partition)
    and x8 (32KB/partition) stay resident in SBUF — pure PE.

DMA discipline (each lesson measured on hardware):
  - >=2KB per-partition lines everywhere: x8 moves in k-QUADS into
    per-half [P, KT, TH] tiles (2KB contiguous), d in k-PAIRS [P, 2, TH]
    (2KB), WP8 in step-pairs (3.5KB). 512B-line transfers ran the sync
    queue at ~100GB/s vs ~285GB/s for the baseline's 2KB lines.
  - Queue separation: sync = x8/d/AT + outs; ACT = WP8/W/BT in k-order.
    Two saturated queues reach ~280GB/s; a queue competing mid-stream
    halves the other's rate.
  - Out writes ride sync (ACT is budgeted for the W stream); gpsimd DMA
    triggers measured ~640ns each — unusable.
PE discipline:
  - Chunks run in pairs with DR steps alternating between the two psum
    banks: back-to-back matmuls on the SAME bank stall ~19ns each
    (measured 234 vs 216ns cadence).
  - A PE idle gap over ~2us drops the clock to 1.2GHz for 3.4us-quantized
    windows (HAM k=4/n=8), so the prologue's DMA-bound slack is spread as
    many sub-0.5us gaps (7-chunk DR burst split 4/3 across k-parity).
  - The pass seam is bridged by the pair (chunk31@h0, chunk0@h1).
"""

import numpy as np

P = 128
D_IN = 4096
D_OUT = 4096
BATCH, SEQ = 4, 2048
TOK = BATCH * SEQ  # 8192
NCORES = 8
T = TOK // NCORES  # 1024 tokens per core
TH = 512  # psum free-dim (= tokens per half)
KT = D_IN // P  # 32 k-tiles
NDR = KT // 2  # 16 DoubleRow steps
NQ = KT // 4  # 8 k-quads (x8 DMA granule)
OC = D_OUT // P  # 32 out chunks of 128
R = 16
NPRO = 7  # out-chunks computed during the pass-A prologue (1 bank each)
DROP_P = 0.05
SCALING = 32.0 / 16.0
WSCALE = 64.0  # fp8 pre-scale for W (exact power of two)

_PROGRAM_CACHE = {}


def _build_program():
    from concourse import bacc
    import concourse.mybir as mybir
    import concourse.tile as tile

    f32 = mybir.dt.float32
    bf16 = mybir.dt.bfloat16
    f8 = mybir.dt.float8e4
    DR = mybir.MatmulPerfMode.DoubleRow

    nc = bacc.Bacc("TRN2", target_bir_lowering=False)
    # x8[h, q, p, u, th] = fp8 x for k-tile 4q+u, half h  (2KB lines)
    x8_d = nc.dram_tensor("x8", [2, NQ, P, 4, TH], f8, kind="ExternalInput")
    # d2[h, s, p, u, th] = bf16 dropout product for k-tile 2s+u  (2KB lines)
    d2_d = nc.dram_tensor("d2", [2, NDR, P, 2, TH], bf16, kind="ExternalInput")
    W8_d = nc.dram_tensor("W8", [OC - NPRO, P, KT, P], f8, kind="ExternalInput")
    # WP2[sp, p, v, j, u, o]: prologue chunks' W for DR steps 2sp+v
    WP2_d = nc.dram_tensor(
        "WP2", [NDR // 2, P, 2, NPRO, 2, P], f8, kind="ExternalInput"
    )
    AT_d = nc.dram_tensor("AT", [P, KT * R], bf16, kind="ExternalInput")
    BT_d = nc.dram_tensor("BT", [OC, R, P], bf16, kind="ExternalInput")
    out_d = nc.dram_tensor("out", [OC, P, T], bf16, kind="ExternalOutput")

    with tile.TileContext(nc) as tc:
        with (
            tc.tile_pool(name="x8", bufs=2) as x8pool,
            tc.tile_pool(name="at", bufs=1) as atpool,
            tc.tile_pool(name="wp", bufs=NDR // 2) as wppool,
            tc.tile_pool(name="wt", bufs=OC - NPRO) as wtpool,
            tc.tile_pool(name="bt", bufs=OC) as btpool,
            tc.tile_pool(name="d", bufs=5) as dpool,
            tc.tile_pool(name="xa", bufs=2) as xapool,
            tc.tile_pool(name="o", bufs=12) as opool,
            tc.tile_pool(name="warm", bufs=2) as warmpool,
            tc.tile_pool(name="ps_pro", bufs=NPRO, space="PSUM") as ps_pro,
            tc.tile_pool(name="ps_xa", bufs=1, space="PSUM") as ps_xa,
        ):
            # per-half resident x8: [P, KT, TH], k-stride TH so a k-quad DMA
            # lands as one contiguous 2KB per-partition line
            x8h_t = [
                x8pool.tile([P, KT, TH], f8, tag="x8", name=f"x8h{h}")
                for h in range(2)
            ]
            AT_t = atpool.tile([P, KT * R], bf16, tag="AT")

            bt_tiles = [
                btpool.tile([R, P], bf16, tag="BT", name=f"BT{c}") for c in range(OC)
            ]
            for c in range(NPRO):
                nc.scalar.dma_start(bt_tiles[c][:], BT_d[c])

            wt_tiles = {}

            def prefetch_wt(c):
                if c < NPRO or c >= OC or c in wt_tiles:
                    return
                wt_t = wtpool.tile([P, KT, P], f8, tag="WT", name=f"WT{c}")
                nc.scalar.dma_start(wt_t[:], W8_d[c - NPRO])
                nc.scalar.dma_start(bt_tiles[c][:], BT_d[c])
                wt_tiles[c] = wt_t

            pro_ps = [
                ps_pro.tile([P, TH], f32, tag="ps", name=f"pps{j}")
                for j in range(NPRO)
            ]
            xa0_ps = ps_xa.tile([R, TH], f32, tag="xa", name="xa0")

            # PE warm-up: the clock governor needs ~3us of continuous busy
            # to reach 2.4GHz, and the first input lands ~9us in. A
            # dependency-free dummy DR chain on memset tiles ramps the
            # clock during otherwise-dead time; the real chunk-0 chain
            # opens with start=True, which resets the bank. Sized so the
            # chain drains just as the first d-pair arrives.
            dw_t = warmpool.tile([P, 2, P], f8, tag="warm", name="warm_w")
            dx_t = warmpool.tile([P, 2, TH], f8, tag="warm", name="warm_x")
            # DVE memsets: gpsimd's took until ~7.9us (engine spin-up),
            # stalling the warm-up chain behind them
            nc.vector.memset(dw_t[:], 0.0)
            nc.vector.memset(dx_t[:], 0.0)
            _warm_first = [True]

            def warm(n, close=False):
                for i in range(n):
                    nc.tensor.matmul(
                        pro_ps[0][:],
                        dw_t[:],
                        dx_t[:],
                        start=_warm_first[0],
                        stop=(close and i == n - 1),
                        perf_mode=DR,
                        skip_group_check=True,
                    )
                    _warm_first[0] = False

            # 20 upfront: the chain must DRAIN just as the first d-pair
            # lands (~14.4us, DMA-queue-bound) — 16 dummies drained at
            # ~12.7us leaving a measured 1.7us idle gap before it
            warm(20)

            wp_tiles = []
            d_tiles = {}

            def dma_xquad(h, q):
                nc.sync.dma_start(x8h_t[h][:, 4 * q : 4 * q + 4, :], x8_d[h, q])

            def dma_dpair(h, s):
                d_t = dpool.tile([P, 2, TH], bf16, tag="d", name=f"d{h}_{s}")
                nc.sync.dma_start(d_t[:], d2_d[h, s])
                d_tiles[(h, s)] = d_t

            def dma_wp(sp):
                if sp >= NDR // 2:
                    return
                wp_t = wppool.tile(
                    [P, 2, NPRO, 2, P], f8, tag="wp", name=f"wp{sp}"
                )
                nc.scalar.dma_start(wp_t[:], WP2_d[sp])
                wp_tiles.append(wp_t)

            def wp_ap(s, j):
                return wp_tiles[s // 2][:, s % 2, j]

            # ---- pass-A prologue: h0 x8/d stream per-k; stage-1(h0) and 7
            # chunks' DR chains (h0 only, one bank each) ride the stream.
            # AT + first d-pair lead the sync queue: stage-1(0) is the
            # first PE op, so its inputs land first.
            nc.sync.dma_start(AT_t[:], AT_d[:])
            dma_dpair(0, 0)
            dma_xquad(0, 0)
            dma_wp(0)
            for k in range(KT):
                if k % 4 == 0 and k // 4 + 1 < NQ:
                    dma_xquad(0, k // 4 + 1)
                if k % 2 == 0 and k // 2 + 1 < NDR:
                    dma_dpair(0, k // 2 + 1)
                if k % 4 == 0:
                    dma_wp(k // 4 + 1)
                if k == 28:
                    prefetch_wt(NPRO)  # first steady chunks' W follow WP2
                    prefetch_wt(NPRO + 1)
                if k >= 1:
                    kk = k - 1
                    nc.tensor.matmul(
                        xa0_ps[:],
                        AT_t[:, kk * R : (kk + 1) * R],
                        d_tiles[(0, kk // 2)][:, kk % 2, :],
                        start=(kk == 0),
                        stop=False,
                    )
                    if kk % 2 == 1:
                        del d_tiles[(0, kk // 2)]
                    if k <= 4:
                        # bridge the first data-pacing gaps while the DMA
                        # stream ramps; the k==4 batch CLOSES the dummy
                        # accumulation group (an open group costs ~58ns on
                        # every later matmul, measured 437 vs 379ns)
                        warm(2, close=(k == 4))
                # split the 7-chunk DR burst 4/3 across k-parity: smaller,
                # more frequent PE bursts keep idle gaps well under the
                # ~2us p-state downshift threshold in the DMA-bound stream
                if k >= 2 and k % 2 == 0:
                    s = (k - 2) // 2
                    for j in range(4):
                        nc.tensor.matmul(
                            pro_ps[j][:],
                            wp_ap(s, j),
                            x8h_t[0][:, k - 2 : k, :],
                            start=(s == 0),
                            stop=False,
                            perf_mode=DR,
                        )
                elif k >= 3:
                    s = (k - 3) // 2
                    for j in range(4, NPRO):
                        nc.tensor.matmul(
                            pro_ps[j][:],
                            wp_ap(s, j),
                            x8h_t[0][:, k - 3 : k - 1, :],
                            start=(s == 0),
                            stop=False,
                            perf_mode=DR,
                        )
            for j in range(4):
                nc.tensor.matmul(
                    pro_ps[j][:],
                    wp_ap(NDR - 1, j),
                    x8h_t[0][:, KT - 2 : KT, :],
                    start=False,
                    stop=False,
                    perf_mode=DR,
                )
            nc.tensor.matmul(
                xa0_ps[:],
                AT_t[:, (KT - 1) * R : KT * R],
                d_tiles[(0, NDR - 1)][:, 1, :],
                start=False,
                stop=True,
            )
            del d_tiles[(0, NDR - 1)]
            for j in range(4, NPRO):
                nc.tensor.matmul(
                    pro_ps[j][:],
                    wp_ap(NDR - 1, j),
                    x8h_t[0][:, KT - 2 : KT, :],
                    start=False,
                    stop=False,
                    perf_mode=DR,
                )

            xa0_t = xapool.tile([R, TH], bf16, tag="xaT", name="xa0t")
            nc.vector.tensor_copy(xa0_t[:], xa0_ps[:])
            xa_ts = [xa0_t, None]

            def drain(c, ps, h):
                # out writes ride the sync queue: in steady-A the ACT queue
                # is budgeted for the W stream alone
                o_t = opool.tile([P, TH], bf16, tag="o", name=f"o{c}_{h}")
                nc.vector.tensor_scalar_mul(o_t[:], ps[:], 1.0 / WSCALE)
                nc.sync.dma_start(out_d[c, :, h * TH : (h + 1) * TH], o_t[:])

            # prologue chunks: stage-2 closes the accumulation
            for j in range(NPRO):
                nc.tensor.matmul(
                    pro_ps[j][:], bt_tiles[j][:], xa0_t[:], start=False, stop=True
                )
                drain(j, pro_ps[j], 0)

            def chain_pair(items):
                # items: list of (chunk, half); DR steps alternate between
                # the pair's psum banks so no two consecutive matmuls hit
                # the same bank.
                pss = [
                    ps_pro.tile([P, TH], f32, tag="ps", name=f"ps{c}_{h}")
                    for c, h in items
                ]
                for (c, h), ps in zip(items, pss):
                    nc.tensor.matmul(
                        ps[:], bt_tiles[c][:], xa_ts[h][:], start=True, stop=False
                    )
                for s in range(NDR):
                    for (c, h), ps in zip(items, pss):
                        if c < NPRO:
                            w_ap = wp_ap(s, c)
                        else:
                            w_ap = wt_tiles[c][:, 2 * s : 2 * s + 2, :]
                        nc.tensor.matmul(
                            ps[:],
                            w_ap,
                            x8h_t[h][:, 2 * s : 2 * s + 2, :],
                            start=False,
                            stop=(s == NDR - 1),
                            perf_mode=DR,
                        )
                for (c, h), ps in zip(items, pss):
                    drain(c, ps, h)

            # ---- pass-A steady: chunks 7..30 in 12 pairs; h1's x8/d stream
            # + stage-1(h1) interleave, paced to finish early so the xa1
            # copy overlaps the last pairs.
            xa1_ps = ps_xa.tile([R, TH], f32, tag="xa", name="xa1")
            xa1_t = xapool.tile([R, TH], bf16, tag="xaT", name="xa1t")
            xa_ts[1] = xa1_t
            sp_dma = 0  # h1 d-pair DMA issue position (one pair ahead)
            kk = 0  # stage-1(h1) matmul position (k units)
            npairs = (OC - 1 - NPRO) // 2  # 12

            def h1_stream_dma(tgt):
                nonlocal sp_dma
                while sp_dma < tgt:
                    if sp_dma % 2 == 0:
                        dma_xquad(1, sp_dma // 2)
                    dma_dpair(1, sp_dma)
                    sp_dma += 1

            def pace(ip):
                # d-pair units; stage-1(h1) done by pair 9 of 12
                return min(NDR, ((ip + 1) * NDR + 9) // 10)

            prefetch_wt(NPRO + 2)
            prefetch_wt(NPRO + 3)
            for ip in range(npairs):
                c0 = NPRO + 2 * ip
                h1_stream_dma(pace(ip + 1))
                target = 2 * pace(ip)
                while kk < target:
                    nc.tensor.matmul(
                        xa1_ps[:],
                        AT_t[:, kk * R : (kk + 1) * R],
                        d_tiles[(1, kk // 2)][:, kk % 2, :],
                        start=(kk == 0),
                        stop=(kk == KT - 1),
                    )
                    if kk % 2 == 1:
                        del d_tiles[(1, kk // 2)]
                    kk += 1
                    if kk == KT:
                        nc.vector.tensor_copy(xa1_t[:], xa1_ps[:])
                prefetch_wt(c0 + 4)
                prefetch_wt(c0 + 5)
                chain_pair([(c0, 0), (c0 + 1, 0)])

            # seam pair bridges pass A -> pass B
            chain_pair([(OC - 1, 0), (0, 1)])

            # ---- pass B: chunks 1..30 in pairs + final chunk 31; zero
            # input DMA (W + x8 resident), pure PE.
            for ip in range(15):
                c0 = 1 + 2 * ip
                chain_pair([(c0, 1), (c0 + 1, 1)])
            chain_pair([(OC - 1, 1)])

    nc.finalize()
    return nc


def _get_program():
    if "nc" not in _PROGRAM_CACHE:
        _PROGRAM_CACHE["nc"] = _build_program()
    return _PROGRAM_CACHE["nc"]


def kernel(hidden_states, W_base, A, B, dropout_mask):
    import ml_dtypes
    from concourse.bass_utils import run_bass_kernel_spmd

    bf = ml_dtypes.bfloat16
    f8 = ml_dtypes.float8_e4m3

    hs = np.ascontiguousarray(np.asarray(hidden_states, dtype=np.float32)).reshape(
        TOK, D_IN
    )
    mask = np.asarray(dropout_mask).reshape(TOK, D_IN)
    W = np.asarray(W_base, dtype=np.float32)
    A_ = np.asarray(A, dtype=np.float32)
    B_ = np.asarray(B, dtype=np.float32)

    #   full[oc, pk, k, o] = W[oc*128+o, k*128+pk] * 64 (fp8 pre-scale)
    Wfull = (W * np.float32(WSCALE)).reshape(OC, P, KT, P).transpose(0, 3, 2, 1)
    W8 = np.ascontiguousarray(Wfull[NPRO:]).astype(f8)
    #   WP2[sp, pk, v, j, u, o] = Wfull[j, pk, 2(2sp+v)+u, o]
    WP2 = np.ascontiguousarray(
        Wfull[:NPRO].reshape(NPRO, P, NDR // 2, 2, 2, P).transpose(2, 1, 3, 0, 4, 5)
    ).astype(f8)
    #   AT[pk, k*16+r] = A[r, k*128+pk] / (1-p)
    AT = (
        np.ascontiguousarray(A_.T.reshape(KT, P, R).transpose(1, 0, 2)).reshape(
            P, KT * R
        )
        * np.float32(1.0 / (1.0 - DROP_P))
    ).astype(bf)
    #   BT[oc, r, o] = B[oc*128+o, r] * scaling * 64
    BT = (
        np.ascontiguousarray(B_.reshape(OC, P, R).transpose(0, 2, 1))
        * np.float32(SCALING * WSCALE)
    ).astype(bf)

    in_maps = []
    for c in range(NCORES):
        sl = slice(c * T, (c + 1) * T)
        #   x8[h, q, p, u, th] = fp8(x[c*T + h*TH + th, (4q+u)*128+p])
        xc = np.ascontiguousarray(hs[sl].T).reshape(KT, P, T)
        x8full = xc.astype(f8)  # [KT, P, T]
        x8 = np.ascontiguousarray(
            x8full.reshape(NQ, 4, P, 2, TH).transpose(3, 0, 2, 1, 4)
        )
        mc = np.ascontiguousarray(mask[sl].T).reshape(KT, P, T)
        dbf = np.where(mc, xc.astype(bf), np.zeros((), dtype=bf))  # [KT, P, T]
        d2 = np.ascontiguousarray(
            dbf.reshape(NDR, 2, P, 2, TH).transpose(3, 0, 2, 1, 4)
        )
        in_maps.append(
            {"x8": x8, "d2": d2, "W8": W8, "WP2": WP2, "AT": AT, "BT": BT}
        )

    nc = _get_program()
    res = run_bass_kernel_spmd(nc, in_maps, core_ids=list(range(NCORES)))
    _PROGRAM_CACHE["last_results"] = res

    # out_dev[oc, o, t] = out[t, oc*128+o]  (per core, bf16 on device)
    parts = []
    for c in range(NCORES):
        od = res.results[c]["out"].astype(np.float32)  # [OC, P, T]
        parts.append(od.reshape(D_OUT, T).T)
    out = np.concatenate(parts, axis=0)
    return np.ascontiguousarray(out.reshape(BATCH, SEQ, D_OUT)).astype(np.float32)


# revision 41
# speedup vs baseline: 1.0253x; 1.0253x over previous
"""LoRA Linear (residual + low-rank path with dropout) on 8 Trainium2 cores.

Math (fp32 reference):
  residual = hidden_states @ W_base.T
  dropped  = hidden_states * dropout_mask / (1 - p)
  out      = residual + ((dropped @ A.T) @ B.T) * scaling

Sharding: data-parallel over the 8192 tokens (8 cores x 1024 tokens);
W_base / A / B replicated.

v12 — token-split two-pass structure. The host ships x pre-cast to
fp8-e4m3 (4MB/core) and the dropout product d = bf16(x)*mask in bf16
(8MB/core, replacing the u8 mask + on-device DVE cast/mult). The residual
matmul runs fp8 DoubleRow (W pre-scaled by 64, drain rescales 1/64); the
LoRA path (A', B', d) stays bf16 — fp8 anywhere on the LoRA path fails
the 2e-2 gate (measured 3.2e-2 in numpy). Output is written bf16 (halves
drain traffic; adds <1e-3 to a ~5e-3 error vs the 2e-2 gate).

The 1024 tokens split into two 512-token halves so each PSUM chain is one
[128, 512] bank:
  Pass A (h0): per-k stream of x8/d rides 7 prologue chunks' DR chains
    (one bank each) + stage-1(h0) (8th bank). Then 24 steady chunks in
    PAIRS; h1's x8/d + stage-1(h1) interleave, DMA one step ahead of the
    consuming matmul.
  Pass B (h1): all 32 chunks with ZERO input DMA — W (100KB/partition)
    and x8 (32KB/partition) stay resident in SBUF — pure PE.

DMA discipline (each lesson measured on hardware):
  - >=2KB per-partition lines everywhere: x8 moves in k-QUADS into
    per-half [P, KT, TH] tiles (2KB contiguous), d in k-PAIRS [P, 2, TH]
    (2KB), WP8 in step-pairs (3.5KB). 512B-line transfers ran the sync
    queue at ~100GB/s vs ~285GB/s for the baseline's 2KB lines.
  - Queue separation: sync = x8/d/AT + outs; ACT = WP8/W/BT in k-order.
    Two saturated queues reach ~280GB/s; a queue competing mid-stream
    halves the other's rate.
  - Out writes ride sync (ACT is budgeted for the W stream); gpsimd DMA
    triggers measured ~640ns each — unusable.
PE discipline:
  - Chunks run in pairs with DR steps alternating between the two psum
    banks: back-to-back matmuls on the SAME bank stall ~19ns each
    (measured 234 vs 216ns cadence).
  - A PE idle gap over ~2us drops the clock to 1.2GHz for 3.4us-quantized
    windows (HAM k=4/n=8), so the prologue's DMA-bound slack is spread as
    many sub-0.5us gaps (7-chunk DR burst split 4/3 across k-parity).
  - The pass seam is bridged by the pair (chunk31@h0, chunk0@h1).
"""

import numpy as np

P = 128
D_IN = 4096
D_OUT = 4096
BATCH, SEQ = 4, 2048
TOK = BATCH * SEQ  # 8192
NCORES = 8
T = TOK // NCORES  # 1024 tokens per core
TH = 512  # psum free-dim (= tokens per half)
KT = D_IN // P  # 32 k-tiles
NDR = KT // 2  # 16 DoubleRow steps
NQ = KT // 4  # 8 k-quads (x8 DMA granule)
OC = D_OUT // P  # 32 out chunks of 128
R = 16
NPRO = 7  # out-chunks computed during the pass-A prologue (1 bank each)
DROP_P = 0.05
SCALING = 32.0 / 16.0
WSCALE = 64.0  # fp8 pre-scale for W (exact power of two)

_PROGRAM_CACHE = {}


def _build_program():
    from concourse import bacc
    import concourse.mybir as mybir
    import concourse.tile as tile

    f32 = mybir.dt.float32
    bf16 = mybir.dt.bfloat16
    f8 = mybir.dt.float8e4
    DR = mybir.MatmulPerfMode.DoubleRow

    nc = bacc.Bacc("TRN2", target_bir_lowering=False)
    # x8[h, q, p, u, th] = fp8 x for k-tile 4q+u, half h  (2KB lines)
    x8_d = nc.dram_tensor("x8", [2, NQ, P, 4, TH], f8, kind="ExternalInput")
    # d2[h, s, p, u, th] = bf16 dropout product for k-tile 2s+u  (2KB lines)
    d2_d = nc.dram_tensor("d2", [2, NDR, P, 2, TH], bf16, kind="ExternalInput")
    W8_d = nc.dram_tensor("W8", [OC - NPRO, P, KT, P], f8, kind="ExternalInput")
    # WP2[sp, p, v, j, u, o]: prologue chunks' W for DR steps 2sp+v
    WP2_d = nc.dram_tensor(
        "WP2", [NDR // 2, P, 2, NPRO, 2, P], f8, kind="ExternalInput"
    )
    AT_d = nc.dram_tensor("AT", [P, KT * R], bf16, kind="ExternalInput")
    BT_d = nc.dram_tensor("BT", [OC, R, P], bf16, kind="ExternalInput")
    out_d = nc.dram_tensor("out", [OC, P, T], bf16, kind="ExternalOutput")

    with tile.TileContext(nc) as tc:
        with (
            tc.tile_pool(name="x8", bufs=2) as x8pool,
            tc.tile_pool(name="at", bufs=1) as atpool,
            tc.tile_pool(name="wp", bufs=NDR // 2) as wppool,
            tc.tile_pool(name="wt", bufs=OC - NPRO) as wtpool,
            tc.tile_pool(name="bt", bufs=OC) as btpool,
            tc.tile_pool(name="d", bufs=5) as dpool,
            tc.tile_pool(name="xa", bufs=2) as xapool,
            tc.tile_pool(name="o", bufs=12) as opool,
            tc.tile_pool(name="warm", bufs=2) as warmpool,
            tc.tile_pool(name="ps_pro", bufs=NPRO, space="PSUM") as ps_pro,
            tc.tile_pool(name="ps_xa", bufs=1, space="PSUM") as ps_xa,
        ):
            # per-half resident x8: [P, KT, TH], k-stride TH so a k-quad DMA
            # lands as one contiguous 2KB per-partition line
            x8h_t = [
                x8pool.tile([P, KT, TH], f8, tag="x8", name=f"x8h{h}")
                for h in range(2)
            ]
            AT_t = atpool.tile([P, KT * R], bf16, tag="AT")

            bt_tiles = [
                btpool.tile([R, P], bf16, tag="BT", name=f"BT{c}") for c in range(OC)
            ]
            for c in range(NPRO):
                nc.scalar.dma_start(bt_tiles[c][:], BT_d[c])

            wt_tiles = {}

            def prefetch_wt(c):
                if c < NPRO or c >= OC or c in wt_tiles:
                    return
                wt_t = wtpool.tile([P, KT, P], f8, tag="WT", name=f"WT{c}")
                nc.scalar.dma_start(wt_t[:], W8_d[c - NPRO])
                nc.scalar.dma_start(bt_tiles[c][:], BT_d[c])
                wt_tiles[c] = wt_t

            pro_ps = [
                ps_pro.tile([P, TH], f32, tag="ps", name=f"pps{j}")
                for j in range(NPRO)
            ]
            xa0_ps = ps_xa.tile([R, TH], f32, tag="xa", name="xa0")

            # PE warm-up: the clock governor needs ~3us of continuous busy
            # to reach 2.4GHz, and the first input lands ~9us in. A
            # dependency-free dummy DR chain on memset tiles ramps the
            # clock during otherwise-dead time; the real chunk-0 chain
            # opens with start=True, which resets the bank. Sized so the
            # chain drains just as the first d-pair arrives.
            dw_t = warmpool.tile([P, 2, P], f8, tag="warm", name="warm_w")
            dx_t = warmpool.tile([P, 2, TH], f8, tag="warm", name="warm_x")
            # DVE memsets: gpsimd's took until ~7.9us (engine spin-up),
            # stalling the warm-up chain behind them
            nc.vector.memset(dw_t[:], 0.0)
            nc.vector.memset(dx_t[:], 0.0)
            _warm_first = [True]

            def warm(n, close=False):
                for i in range(n):
                    nc.tensor.matmul(
                        pro_ps[0][:],
                        dw_t[:],
                        dx_t[:],
                        start=_warm_first[0],
                        stop=(close and i == n - 1),
                        perf_mode=DR,
                        skip_group_check=True,
                    )
                    _warm_first[0] = False

            # 16 upfront: the chain drains just before the first d-pair
            # lands (~14.4us, DMA-queue-bound); 16 measured best (286.2us)
            # vs 20 (291.8us) and 24 (287.1us) at the same clock tier
            warm(16)

            wp_tiles = []
            d_tiles = {}

            def dma_xquad(h, q):
                nc.sync.dma_start(x8h_t[h][:, 4 * q : 4 * q + 4, :], x8_d[h, q])

            def dma_dpair(h, s):
                d_t = dpool.tile([P, 2, TH], bf16, tag="d", name=f"d{h}_{s}")
                nc.sync.dma_start(d_t[:], d2_d[h, s])
                d_tiles[(h, s)] = d_t

            def dma_wp(sp):
                if sp >= NDR // 2:
                    return
                wp_t = wppool.tile(
                    [P, 2, NPRO, 2, P], f8, tag="wp", name=f"wp{sp}"
                )
                nc.scalar.dma_start(wp_t[:], WP2_d[sp])
                wp_tiles.append(wp_t)

            def wp_ap(s, j):
                return wp_tiles[s // 2][:, s % 2, j]

            # ---- pass-A prologue: h0 x8/d stream per-k; stage-1(h0) and 7
            # chunks' DR chains (h0 only, one bank each) ride the stream.
            # AT + first d-pair lead the sync queue: stage-1(0) is the
            # first PE op, so its inputs land first.
            nc.sync.dma_start(AT_t[:], AT_d[:])
            dma_dpair(0, 0)
            dma_xquad(0, 0)
            dma_wp(0)
            for k in range(KT):
                if k % 4 == 0 and k // 4 + 1 < NQ:
                    dma_xquad(0, k // 4 + 1)
                if k % 2 == 0 and k // 2 + 1 < NDR:
                    dma_dpair(0, k // 2 + 1)
                if k % 4 == 0:
                    dma_wp(k // 4 + 1)
                if k == 28:
                    prefetch_wt(NPRO)  # first steady chunks' W follow WP2
                    prefetch_wt(NPRO + 1)
                if k >= 1:
                    kk = k - 1
                    nc.tensor.matmul(
                        xa0_ps[:],
                        AT_t[:, kk * R : (kk + 1) * R],
                        d_tiles[(0, kk // 2)][:, kk % 2, :],
                        start=(kk == 0),
                        stop=False,
                    )
                    if kk % 2 == 1:
                        del d_tiles[(0, kk // 2)]
                    if k <= 4:
                        # bridge the first data-pacing gaps while the DMA
                        # stream ramps; the k==4 batch CLOSES the dummy
                        # accumulation group (an open group costs ~58ns on
                        # every later matmul, measured 437 vs 379ns)
                        warm(2, close=(k == 4))
                # split the 7-chunk DR burst 4/3 across k-parity: smaller,
                # more frequent PE bursts keep idle gaps well under the
                # ~2us p-state downshift threshold in the DMA-bound stream
                if k >= 2 and k % 2 == 0:
                    s = (k - 2) // 2
                    for j in range(4):
                        nc.tensor.matmul(
                            pro_ps[j][:],
                            wp_ap(s, j),
                            x8h_t[0][:, k - 2 : k, :],
                            start=(s == 0),
                            stop=False,
                            perf_mode=DR,
                        )
                elif k >= 3:
                    s = (k - 3) // 2
                    for j in range(4, NPRO):
                        nc.tensor.matmul(
                            pro_ps[j][:],
                            wp_ap(s, j),
                            x8h_t[0][:, k - 3 : k - 1, :],
                            start=(s == 0),
                            stop=False,
                            perf_mode=DR,
                        )
            for j in range(4):
                nc.tensor.matmul(
                    pro_ps[j][:],
                    wp_ap(NDR - 1, j),
                    x8h_t[0][:, KT - 2 : KT, :],
                    start=False,
                    stop=False,
                    perf_mode=DR,
                )
            nc.tensor.matmul(
                xa0_ps[:],
                AT_t[:, (KT - 1) * R : KT * R],
                d_tiles[(0, NDR - 1)][:, 1, :],
                start=False,
                stop=True,
            )
            del d_tiles[(0, NDR - 1)]
            for j in range(4, NPRO):
                nc.tensor.matmul(
                    pro_ps[j][:],
                    wp_ap(NDR - 1, j),
                    x8h_t[0][:, KT - 2 : KT, :],
                    start=False,
                    stop=False,
                    perf_mode=DR,
                )

            xa0_t = xapool.tile([R, TH], bf16, tag="xaT", name="xa0t")
            nc.vector.tensor_copy(xa0_t[:], xa0_ps[:])
            xa_ts = [xa0_t, None]

            def drain(c, ps, h):
                # out writes ride the sync queue: in steady-A the ACT queue
                # is budgeted for the W stream alone
                o_t = opool.tile([P, TH], bf16, tag="o", name=f"o{c}_{h}")
                nc.vector.tensor_scalar_mul(o_t[:], ps[:], 1.0 / WSCALE)
                nc.sync.dma_start(out_d[c, :, h * TH : (h + 1) * TH], o_t[:])

            # prologue chunks: stage-2 closes the accumulation
            for j in range(NPRO):
                nc.tensor.matmul(
                    pro_ps[j][:], bt_tiles[j][:], xa0_t[:], start=False, stop=True
                )
                drain(j, pro_ps[j], 0)

            def chain_pair(items):
                # items: list of (chunk, half); DR steps alternate between
                # the pair's psum banks so no two consecutive matmuls hit
                # the same bank.
                pss = [
                    ps_pro.tile([P, TH], f32, tag="ps", name=f"ps{c}_{h}")
                    for c, h in items
                ]
                for (c, h), ps in zip(items, pss):
                    nc.tensor.matmul(
                        ps[:], bt_tiles[c][:], xa_ts[h][:], start=True, stop=False
                    )
                for s in range(NDR):
                    for (c, h), ps in zip(items, pss):
                        if c < NPRO:
                            w_ap = wp_ap(s, c)
                        else:
                            w_ap = wt_tiles[c][:, 2 * s : 2 * s + 2, :]
                        nc.tensor.matmul(
                            ps[:],
                            w_ap,
                            x8h_t[h][:, 2 * s : 2 * s + 2, :],
                            start=False,
                            stop=(s == NDR - 1),
                            perf_mode=DR,
                        )
                for (c, h), ps in zip(items, pss):
                    drain(c, ps, h)

            # ---- pass-A steady: chunks 7..30 in 12 pairs; h1's x8/d stream
            # + stage-1(h1) interleave, paced to finish early so the xa1
            # copy overlaps the last pairs.
            xa1_ps = ps_xa.tile([R, TH], f32, tag="xa", name="xa1")
            xa1_t = xapool.tile([R, TH], bf16, tag="xaT", name="xa1t")
            xa_ts[1] = xa1_t
            sp_dma = 0  # h1 d-pair DMA issue position (one pair ahead)
            kk = 0  # stage-1(h1) matmul position (k units)
            npairs = (OC - 1 - NPRO) // 2  # 12

            def h1_stream_dma(tgt):
                nonlocal sp_dma
                while sp_dma < tgt:
                    if sp_dma % 2 == 0:
                        dma_xquad(1, sp_dma // 2)
                    dma_dpair(1, sp_dma)
                    sp_dma += 1

            def pace(ip):
                # d-pair units; stage-1(h1) done by pair 9 of 12
                return min(NDR, ((ip + 1) * NDR + 9) // 10)

            prefetch_wt(NPRO + 2)
            prefetch_wt(NPRO + 3)
            for ip in range(npairs):
                c0 = NPRO + 2 * ip
                h1_stream_dma(pace(ip + 1))
                target = 2 * pace(ip)
                while kk < target:
                    nc.tensor.matmul(
                        xa1_ps[:],
                        AT_t[:, kk * R : (kk + 1) * R],
                        d_tiles[(1, kk // 2)][:, kk % 2, :],
                        start=(kk == 0),
                        stop=(kk == KT - 1),
                    )
                    if kk % 2 == 1:
                        del d_tiles[(1, kk // 2)]
                    kk += 1
                    if kk == KT:
                        nc.vector.tensor_copy(xa1_t[:], xa1_ps[:])
                prefetch_wt(c0 + 4)
                prefetch_wt(c0 + 5)
                chain_pair([(c0, 0), (c0 + 1, 0)])

            # seam pair bridges pass A -> pass B
            chain_pair([(OC - 1, 0), (0, 1)])

            # ---- pass B: chunks 1..28 in QUADS (4-way psum-bank rotation
            # amortizes chain-boundary bubbles, ~3ns/matmul measured),
            # then a pair + final chunk 31; zero input DMA (W + x8
            # resident), pure PE.
            for iq in range(7):
                c0 = 1 + 4 * iq
                chain_pair([(c0 + i, 1) for i in range(4)])
            chain_pair([(29, 1), (30, 1)])
            chain_pair([(OC - 1, 1)])

    nc.finalize()
    return nc


def _get_program():
    if "nc" not in _PROGRAM_CACHE:
        _PROGRAM_CACHE["nc"] = _build_program()
    return _PROGRAM_CACHE["nc"]


def kernel(hidden_states, W_base, A, B, dropout_mask):
    import ml_dtypes
    from concourse.bass_utils import run_bass_kernel_spmd

    bf = ml_dtypes.bfloat16
    f8 = ml_dtypes.float8_e4m3

    hs = np.ascontiguousarray(np.asarray(hidden_states, dtype=np.float32)).reshape(
        TOK, D_IN
    )
    mask = np.asarray(dropout_mask).reshape(TOK, D_IN)
    W = np.asarray(W_base, dtype=np.float32)
    A_ = np.asarray(A, dtype=np.float32)
    B_ = np.asarray(B, dtype=np.float32)

    #   full[oc, pk, k, o] = W[oc*128+o, k*128+pk] * 64 (fp8 pre-scale)
    Wfull = (W * np.float32(WSCALE)).reshape(OC, P, KT, P).transpose(0, 3, 2, 1)
    W8 = np.ascontiguousarray(Wfull[NPRO:]).astype(f8)
    #   WP2[sp, pk, v, j, u, o] = Wfull[j, pk, 2(2sp+v)+u, o]
    WP2 = np.ascontiguousarray(
        Wfull[:NPRO].reshape(NPRO, P, NDR // 2, 2, 2, P).transpose(2, 1, 3, 0, 4, 5)
    ).astype(f8)
    #   AT[pk, k*16+r] = A[r, k*128+pk] / (1-p)
    AT = (
        np.ascontiguousarray(A_.T.reshape(KT, P, R).transpose(1, 0, 2)).reshape(
            P, KT * R
        )
        * np.float32(1.0 / (1.0 - DROP_P))
    ).astype(bf)
    #   BT[oc, r, o] = B[oc*128+o, r] * scaling * 64
    BT = (
        np.ascontiguousarray(B_.reshape(OC, P, R).transpose(0, 2, 1))
        * np.float32(SCALING * WSCALE)
    ).astype(bf)

    in_maps = []
    for c in range(NCORES):
        sl = slice(c * T, (c + 1) * T)
        #   x8[h, q, p, u, th] = fp8(x[c*T + h*TH + th, (4q+u)*128+p])
        xc = np.ascontiguousarray(hs[sl].T).reshape(KT, P, T)
        x8full = xc.astype(f8)  # [KT, P, T]
        x8 = np.ascontiguousarray(
            x8full.reshape(NQ, 4, P, 2, TH).transpose(3, 0, 2, 1, 4)
        )
        mc = np.ascontiguousarray(mask[sl].T).reshape(KT, P, T)
        dbf = np.where(mc, xc.astype(bf), np.zeros((), dtype=bf))  # [KT, P, T]
        d2 = np.ascontiguousarray(
            dbf.reshape(NDR, 2, P, 2, TH).transpose(3, 0, 2, 1, 4)
        )
        in_maps.append(
            {"x8": x8, "d2": d2, "W8": W8, "WP2": WP2, "AT": AT, "BT": BT}
        )

    nc = _get_program()
    res = run_bass_kernel_spmd(nc, in_maps, core_ids=list(range(NCORES)))
    _PROGRAM_CACHE["last_results"] = res

    # out_dev[oc, o, t] = out[t, oc*128+o]  (per core, bf16 on device)
    parts = []
    for c in range(NCORES):
        od = res.results[c]["out"].astype(np.float32)  # [OC, P, T]
        parts.append(od.reshape(D_OUT, T).T)
    out = np.concatenate(parts, axis=0)
    return np.ascontiguousarray(out.reshape(BATCH, SEQ, D_OUT)).astype(np.float32)
